# revision 2
# baseline (speedup 1.0000x reference)
"""Trainium2 Bass kernel for nn_NeighborhoodSelfAttentionBlock.

Strategy (8 NeuronCores, single launch, SPMD):
  - Shard the T axis: core c computes the output for T-plane c (256 tokens).
  - Each core redundantly preprocesses + projects qkv for its 3-plane halo
    (clamped NATTEN window), so no cross-core communication is needed.
  - BitLinear is computed exactly: int8-grid activations and ternary weights
    are exact in bf16; the matmul accumulates exact integers in f32 PSUM.
    Rounding uses the f32 magic-number trick (round-half-even == jnp.round).
  - Cosine-sim attention is scale invariant, so q/k stay in integer scale
    until normalization; softmax needs no max-subtraction (|logits| <= 10).
  - 3D neighborhood attention: 4-row query strips x (3 t-planes) key blocks,
    block-dense logits in L^T layout (keys on partitions) with host-built
    masks applied multiplicatively after exp; denominator via a ones column
    appended to v.
  - rsqrt is computed as exp(-0.5*ln(x)) so the single ACT table set
    natural_log_exp_and_others covers every activation in the kernel.
"""

import math
import os
import sys

import numpy as np

sys.path.insert(0, "/opt/trn_rl_repo")

import ml_dtypes

BF16 = ml_dtypes.bfloat16
F16 = np.float16

D = 512
NH = 8
DH = 64
KT, KH, KW = 3, 5, 5
T, H, W = 8, 16, 16
NTOK = T * H * W
PLANE = H * W  # 256
MAGIC = float(np.float32(1.5 * 2 ** 23))
EPS = 1e-6

# attention strip geometry: 4 query h-rows per strip; key blocks are the
# half-planes (128 tokens) overlapping the strip's h-window, per t-plane.
HALVES = [[0], [0, 1], [0, 1], [1]]

_CACHE = {}


def _win_starts(n, k):
    return np.clip(np.arange(n) - k // 2, 0, n - k)


def _make_masks():
    hs = _win_starts(H, KH)
    ws = _win_starts(W, KW)
    big = np.zeros((2, 128, 192), np.float16)
    for eta in range(2):
        strips = [0, 1, 2] if eta == 0 else [1, 2, 3]
        for si, s in enumerate(strips):
            for i, h in enumerate(range(4 * s, 4 * s + 4)):
                for w in range(W):
                    for hk in range(hs[h], hs[h] + KH):
                        if not (8 * eta <= hk < 8 * eta + 8):
                            continue
                        for wk in range(ws[w], ws[w] + KW):
                            big[eta, (hk - 8 * eta) * W + wk,
                                si * 64 + i * W + w] = 1.0
    return big


def _rope_tables(pos):
    dim = DH // 4
    npgh = dim // 4
    freqs = np.exp(
        np.linspace(math.log(math.pi), math.log(10 * math.pi), NH * npgh + 1)[:-1]
    )
    freqs = freqs.reshape(npgh, NH).T  # (8, 4)
    theta = np.concatenate(
        [pos[:, None, a : a + 1] * freqs[None, :, :] for a in range(3)], axis=-1
    ).astype(np.float32)  # (tok, 8, 12)
    cos, sin = np.cos(theta), np.sin(theta)
    cs2 = np.concatenate([cos, cos], axis=-1).astype(F16)  # (tok, 8, 24)
    sn2 = np.concatenate([-sin, sin], axis=-1).astype(F16)
    return cs2.reshape(NTOK, NH * 24), sn2.reshape(NTOK, NH * 24)


def _make_bacc_class():
    import bass_rust as _bass_rust
    import concourse.bacc as bacc
    from concourse import mybir
    from concourse.hw_specs import get_activation_tables

    class _Bacc(bacc.Bacc):
        """Bacc that pins every activation to natural_log_exp_and_others
        (covers exp/ln/square/copy/identity) so only one ACT table load is
        emitted instead of thrashing between per-function default sets."""

        _KEEP = "natural_log_exp_and_others"

        def insert_act_table_loads(self):
            has_activation = any(
                isinstance(i, mybir.InstActivation)
                for b in self.main_func.blocks
                for i in b.instructions
            )
            if not has_activation:
                return
            used = {
                i.func
                for b in self.main_func.blocks
                for i in b.instructions
                if isinstance(i, mybir.InstActivation)
            }
            all_tables = get_activation_tables(self.m.arch)
            keep_fns = all_tables.get(self._KEEP, set())
            subtract = used & keep_fns
            tables = []
            for name, fns in all_tables.items():
                if name != self._KEEP:
                    fns = fns - subtract
                tables.append((name, fns))
            _bass_rust.insert_act_table_loads(self, tables)

    return _Bacc


def _build_program():
    import concourse.bacc as bacc
    import concourse.bass as bass
    import concourse.tile as tile
    from concourse import mybir

    f32, f16, bf16 = mybir.dt.float32, mybir.dt.float16, mybir.dt.bfloat16
    AX = mybir.AxisListType
    ALU = mybir.AluOpType
    ACTF = mybir.ActivationFunctionType

    nc = _make_bacc_class()("TRN2", target_bir_lowering=False, debug=False, num_devices=8)

    # ---- DRAM I/O ----
    d_xh = nc.dram_tensor("xh", [3 * PLANE, D], f16, kind="ExternalInput")
    d_xo = nc.dram_tensor("xo", [PLANE, D], f16, kind="ExternalInput")
    d_csh = nc.dram_tensor("csh", [3 * PLANE, NH * 24], f16, kind="ExternalInput")
    d_snh = nc.dram_tensor("snh", [3 * PLANE, NH * 24], f16, kind="ExternalInput")
    d_cso = nc.dram_tensor("cso", [PLANE, NH * 24], f16, kind="ExternalInput")
    d_sno = nc.dram_tensor("sno", [PLANE, NH * 24], f16, kind="ExternalInput")
    d_msk = nc.dram_tensor("msk", [2, 128, 192], f16, kind="ExternalInput")
    d_wkv = nc.dram_tensor("wkv", [D, 1024], bf16, kind="ExternalInput")
    d_wq = nc.dram_tensor("wq", [D, 512], bf16, kind="ExternalInput")
    d_wo = nc.dram_tensor("wo", [D, 512], bf16, kind="ExternalInput")
    d_adwt = nc.dram_tensor("adwt", [D, D], f16, kind="ExternalInput")
    d_cnd = nc.dram_tensor("cnd", [128, 4], f16, kind="ExternalInput")
    d_scl = nc.dram_tensor("scl", [1, NH], f32, kind="ExternalInput")
    d_kon = nc.dram_tensor("kon", [1, 2], f32, kind="ExternalInput")
    d_y = nc.dram_tensor("y", [PLANE, D], f32, kind="ExternalOutput")

    from contextlib import ExitStack
    with tile.TileContext(nc) as tc, ExitStack() as ctx:
        consts = ctx.enter_context(tc.tile_pool(name="consts", bufs=1))
        wpool = ctx.enter_context(tc.tile_pool(name="wpool", bufs=1))
        xpool = ctx.enter_context(tc.tile_pool(name="xpool", bufs=3))
        xapool = ctx.enter_context(tc.tile_pool(name="xapool", bufs=8))
        scratch = ctx.enter_context(tc.tile_pool(name="scratch", bufs=3))
        stats = ctx.enter_context(tc.tile_pool(name="stats", bufs=1))
        xqpool = ctx.enter_context(tc.tile_pool(name="xqpool", bufs=3))
        persist = ctx.enter_context(tc.tile_pool(name="persist", bufs=1))
        kqpool = ctx.enter_context(tc.tile_pool(name="kqpool", bufs=3))
        attp = ctx.enter_context(tc.tile_pool(name="attp", bufs=24))
        small = ctx.enter_context(tc.tile_pool(name="small", bufs=4))
        ypool = ctx.enter_context(tc.tile_pool(name="ypool", bufs=2))
        psA = ctx.enter_context(tc.tile_pool(name="psA", bufs=2, space="PSUM"))
        psL = ctx.enter_context(tc.tile_pool(name="psL", bufs=2, space="PSUM"))
        psO = ctx.enter_context(tc.tile_pool(name="psO", bufs=2, space="PSUM"))

        # ---- constants / weights ----
        adas_bc = consts.tile([128, D], f32)
        scale_bc = consts.tile([128, NH], f32)
        kon_bc = consts.tile([128, 2], f32)
        masks_t = consts.tile([128, 2, 192], f16)
        eps_ap = consts.tile([128, 1], f32)
        cnd_t = consts.tile([128, 4], f16)
        ones_t = consts.tile([1, 128], f16)
        nc.vector.memset(eps_ap, EPS)
        nc.vector.memset(ones_t, 1.0)
        from concourse.masks import make_identity
        ident_bf = consts.tile([128, 128], bf16)
        make_identity(nc, ident_bf)
        ident_f16 = consts.tile([128, 128], f16)
        make_identity(nc, ident_f16)

        def pe_transpose(src, dstT, col, ident, dt16):
            # dstT[:, j, col:col+128] <- src[:, j*128:(j+1)*128].T  (PE route)
            for j in range(4):
                pt = psL.tile([128, 128], dt16, tag="pL")
                nc.tensor.transpose(pt, src[:, j * 128 : (j + 1) * 128], ident)
                if j % 2 == 0:
                    nc.vector.tensor_copy(out=dstT[:, j, col : col + 128], in_=pt)
                else:
                    nc.scalar.copy(out=dstT[:, j, col : col + 128], in_=pt)
        nc.sync.dma_start(out=scale_bc, in_=d_scl.ap().broadcast_to([128, NH]))
        nc.sync.dma_start(out=kon_bc, in_=d_kon.ap().broadcast_to([128, 2]))
        nc.sync.dma_start(out=masks_t, in_=d_msk.ap().rearrange("s p q -> p s q"))
        nc.sync.dma_start(out=cnd_t, in_=d_cnd[:, :])

        wkv4 = wpool.tile([128, 4, 1024], bf16)
        wq4 = wpool.tile([128, 4, 512], bf16)
        wo4 = wpool.tile([128, 4, 512], bf16)
        adwt4 = wpool.tile([128, 4, 512], f16)
        nc.sync.dma_start(out=wkv4, in_=d_wkv.ap().rearrange("(j p) n -> p j n", p=128))
        nc.sync.dma_start(out=wq4, in_=d_wq.ap().rearrange("(j p) n -> p j n", p=128))
        nc.sync.dma_start(out=wo4, in_=d_wo.ap().rearrange("(j p) n -> p j n", p=128))
        nc.sync.dma_start(out=adwt4, in_=d_adwt.ap().rearrange("(j p) n -> p j n", p=128))

        csh_t = persist.tile([128, 6, NH * 24], f16)
        snh_t = persist.tile([128, 6, NH * 24], f16)
        cso_t = persist.tile([128, 2, NH * 24], f16)
        sno_t = persist.tile([128, 2, NH * 24], f16)
        nc.sync.dma_start(out=csh_t, in_=d_csh.ap().rearrange("(i p) n -> p i n", p=128))
        nc.sync.dma_start(out=snh_t, in_=d_snh.ap().rearrange("(i p) n -> p i n", p=128))
        nc.sync.dma_start(out=cso_t, in_=d_cso.ap().rearrange("(i p) n -> p i n", p=128))
        nc.sync.dma_start(out=sno_t, in_=d_sno.ap().rearrange("(i p) n -> p i n", p=128))

        # ---- adas = cond @ ada_w.T + 1, broadcast to 128 partitions ----
        pad = psO.tile([1, D], f32, tag="pO")
        for j in range(4):
            nc.tensor.matmul(pad, lhsT=cnd_t[:, j : j + 1], rhs=adwt4[:, j, :],
                             start=(j == 0), stop=(j == 3))
        ad1 = small.tile([1, D], f16)
        nc.scalar.activation(out=ad1, in_=pad, func=ACTF.Identity, bias=1.0, scale=1.0)
        pad2 = psO.tile([128, D], f32, tag="pO")
        nc.tensor.matmul(pad2, lhsT=ones_t, rhs=ad1, start=True, stop=True)
        nc.vector.tensor_copy(out=adas_bc, in_=pad2)

        # ---- preprocess + quantize x (6 halo tiles, then 2 own tiles) ----
        ss_all = stats.tile([128, 8], f32)
        am_all = stats.tile([128, 8], f32)
        xa_tiles = []
        for i in range(8):
            own = i >= 6
            src = d_xo if own else d_xh
            row = (i - 6) * 128 if own else i * 128
            xt = xpool.tile([128, D], f16, tag="xt")
            nc.sync.dma_start(out=xt, in_=src[row : row + 128, :])
            sq = scratch.tile([128, D], f16, tag="sq")
            nc.scalar.activation(out=sq, in_=xt, func=ACTF.Square,
                                 accum_out=ss_all[:, i : i + 1])
            xa = xapool.tile([128, D], f16, tag="xa")
            nc.gpsimd.tensor_mul(xa, xt, adas_bc)
            nc.vector.reduce_max(out=am_all[:, i : i + 1], in_=xa, axis=AX.X,
                                 apply_absolute_value=True)
            xa_tiles.append(xa)

        lnv = stats.tile([128, 8], f32)
        nc.scalar.activation(out=lnv, in_=ss_all, func=ACTF.Ln, bias=eps_ap,
                             scale=1.0 / D)
        rstd = stats.tile([128, 8], f32)
        nc.scalar.activation(out=rstd, in_=lnv, func=ACTF.Exp, bias=0.0, scale=-0.5)
        hm = stats.tile([128, 8], f32)
        nc.vector.tensor_mul(hm, rstd, am_all)
        nc.vector.tensor_scalar_max(out=hm, in0=hm, scalar1=1e-5)
        vs_all = stats.tile([128, 8], f32)
        nc.vector.tensor_scalar(out=vs_all, in0=hm, scalar1=kon_bc[:, 0:1],
                                scalar2=None, op0=ALU.mult)
        hm127 = stats.tile([128, 8], f32)
        nc.vector.tensor_scalar_mul(out=hm127, in0=hm, scalar1=1.0 / 127.0)
        rec = stats.tile([128, 8], f32)
        nc.vector.reciprocal(out=rec, in_=hm127)
        cq_all = stats.tile([128, 8], f32)
        nc.vector.tensor_mul(cq_all, rec, rstd)

        xqT = persist.tile([128, 4, 6 * 128], bf16)   # halo x_q^T
        xqoT = persist.tile([128, 4, 2 * 128], bf16)  # own  x_q^T
        for i in range(8):
            own = i >= 6
            qsc = scratch.tile([128, D], f32, tag="qsc")
            nc.vector.tensor_scalar(out=qsc, in0=xa_tiles[i],
                                    scalar1=cq_all[:, i : i + 1], scalar2=MAGIC,
                                    op0=ALU.mult, op1=ALU.add)
            xq = xqpool.tile([128, D], bf16, tag="xq")
            nc.vector.tensor_scalar_add(out=xq, in0=qsc, scalar1=-MAGIC)
            dstT = xqoT if own else xqT
            col = (i - 6) * 128 if own else i * 128
            pe_transpose(xq, dstT, col, ident_bf, bf16)

        # ---- kv projection + k/v postprocessing (6 halo chunks) ----
        kT = persist.tile([128, 4, 6 * 128], f16)
        v_sb = persist.tile([128, 6, NH * 65], f16)
        # ones columns for the denominator
        nc.vector.memset(v_sb, 1.0)

        def rope_norm(psum, i, cs_t, sn_t, dstT, dst_col, is_q):
            """psum [128,512] int-valued q/k; rope + normalize -> dstT via DMA-T."""
            z = kqpool.tile([128, NH, DH], f16, tag="z")
            # pass-through dims 24:64
            nc.scalar.activation(out=z[:, :, 24:DH],
                                 in_=psum.rearrange("p (h d) -> p h d", h=NH)[:, :, 24:DH],
                                 func=ACTF.Copy)
            rot = psum.rearrange("p (h d) -> p h d", h=NH)[:, :, 0:24]
            m1 = kqpool.tile([128, NH, 24], f16, tag="m1")
            nc.vector.tensor_mul(m1, rot, cs_t[:, i, :].rearrange("p (h d) -> p h d", h=NH))
            swap = bass.AP(tensor=rot.tensor, offset=rot.offset + 12,
                           ap=[list(rot.ap[0]), list(rot.ap[1]), [-12, 2], [1, 12]])
            m2 = kqpool.tile([128, NH, 2, 12], f16, tag="m2")
            nc.vector.tensor_mul(
                m2, swap,
                sn_t[:, i, :].rearrange("p (h two tw) -> p h two tw", h=NH, two=2))
            m2 = m2[:, :, :, :].rearrange("p h two tw -> p h (two tw)")
            nc.vector.tensor_add(z[:, :, 0:24], m1, m2)
            # norms per (token, head)
            zsq = scratch.tile([128, NH, DH], f32, tag="zsq")
            nc.vector.tensor_mul(zsq, z, z)
            ssz = small.tile([128, NH], f32, tag="ssz")
            nc.vector.reduce_sum(out=ssz, in_=zsq, axis=AX.X)
            lnz = small.tile([128, NH], f32, tag="lnz")
            nc.scalar.activation(out=lnz, in_=ssz, func=ACTF.Ln, bias=eps_ap, scale=1.0)
            rs0 = small.tile([128, NH], f32, tag="rs0")
            nc.scalar.activation(out=rs0, in_=lnz, func=ACTF.Exp, bias=0.0, scale=-0.5)
            if is_q:
                nc.vector.tensor_mul(rs0, rs0, scale_bc)
            rs16 = small.tile([128, NH], f16, tag="rs16")
            nc.vector.tensor_copy(out=rs16, in_=rs0)
            zn = kqpool.tile([128, NH, DH], f16, tag="zn")
            nc.vector.tensor_mul(zn, z, rs16[:, :, None].broadcast_to([128, NH, DH]))
            znf = zn.rearrange("p h d -> p (h d)")
            pe_transpose(znf, dstT, dst_col, ident_f16, f16)

        for i in range(6):
            pk = psA.tile([128, 512], f32, tag="pk")
            for j in range(4):
                nc.tensor.matmul(pk, lhsT=xqT[:, j, i * 128 : (i + 1) * 128],
                                 rhs=wkv4[:, j, 0:512], start=(j == 0), stop=(j == 3))
            pv = psA.tile([128, 512], f32, tag="pv")
            for j in range(4):
                nc.tensor.matmul(pv, lhsT=xqT[:, j, i * 128 : (i + 1) * 128],
                                 rhs=wkv4[:, j, 512:1024], start=(j == 0), stop=(j == 3))
            rope_norm(pk, i, csh_t, snh_t, kT, i * 128, is_q=False)
            nc.scalar.activation(
                out=v_sb[:, i, :].rearrange("p (h d) -> p h d", h=NH)[:, :, 0:DH],
                in_=pv.rearrange("p (h d) -> p h d", h=NH),
                func=ACTF.Copy, scale=vs_all[:, i : i + 1])

        # ---- q projection + postprocessing (2 own chunks) ----
        qnT = persist.tile([128, 4, 2 * 128], f16)
        for i in range(2):
            pq = psA.tile([128, 512], f32, tag="pk")
            for j in range(4):
                nc.tensor.matmul(pq, lhsT=xqoT[:, j, i * 128 : (i + 1) * 128],
                                 rhs=wq4[:, j, :], start=(j == 0), stop=(j == 3))
            rope_norm(pq, i, cso_t, sno_t, qnT, i * 128, is_q=True)

        # ---- neighborhood attention ----
        # Batched QK: one matmul per (head, t-plane, half-plane) covering the
        # 3 query strips that use that key block (contiguous query columns).
        o_all = persist.tile([128, 2, D], f16)
        for half in range(2):
            PTs = {}
            for hh in range(4):
                h = half * 4 + hh
                hp, hc = 64 * (h % 2), h // 2
                for ti in range(3):
                    for eta in range(2):
                        pLt = psL.tile([128, 192], f32, tag="pL")
                        nc.tensor.matmul(
                            pLt,
                            lhsT=kT[hp : hp + 64, hc,
                                    ti * 256 + eta * 128 : ti * 256 + eta * 128 + 128],
                            rhs=qnT[hp : hp + 64, hc, eta * 64 : eta * 64 + 192],
                            start=True, stop=True)
                        PT = attp.tile([128, 192], f16, tag="PT")
                        nc.scalar.activation(out=PT, in_=pLt, func=ACTF.Exp)
                        nc.vector.tensor_mul(PT, PT, masks_t[:, eta, :])
                        PTs[(hh, ti, eta)] = PT
            for s in range(4):
                pO = psO.tile([64, 4 * 65], f32, tag="pO")
                for hh in range(4):
                    h = half * 4 + hh
                    blocks = [(ti, eta) for ti in range(3) for eta in HALVES[s]]
                    for bi, (ti, eta) in enumerate(blocks):
                        qoff = (s - eta) * 64
                        nc.tensor.matmul(
                            pO[:, hh * 65 : (hh + 1) * 65],
                            lhsT=PTs[(hh, ti, eta)][:, qoff : qoff + 64],
                            rhs=v_sb[:, 2 * ti + eta, h * 65 : (h + 1) * 65],
                            start=(bi == 0), stop=(bi == len(blocks) - 1))
                recd = small.tile([64, 4], f32, tag="recd")
                den = bass.AP(tensor=pO.tensor, offset=pO.offset + 64,
                              ap=[list(pO.ap[0]), [65, 4]])
                nc.vector.reciprocal(out=recd, in_=den)
                num = bass.AP(tensor=pO.tensor, offset=pO.offset,
                              ap=[list(pO.ap[0]), [65, 4], [1, 64]])
                nc.vector.tensor_mul(
                    o_all[(s % 2) * 64 : (s % 2) * 64 + 64, s // 2,
                          half * 256 : half * 256 + 256].rearrange(
                              "p (a b) -> p a b", a=4),
                    num, recd[:, :, None].broadcast_to([64, 4, 64]))

        # ---- out projection (BitLinear) + residual ----
        oqT = persist.tile([128, 4, 2 * 128], bf16)
        osc_all = stats.tile([128, 2], f32)
        for tt in range(2):
            amo = small.tile([128, 1], f32, tag="amo")
            nc.vector.reduce_max(out=amo, in_=o_all[:, tt, :], axis=AX.X,
                                 apply_absolute_value=True)
            nc.vector.tensor_scalar_max(out=amo, in0=amo, scalar1=1e-5)
            nc.vector.tensor_scalar(out=osc_all[:, tt : tt + 1], in0=amo,
                                    scalar1=kon_bc[:, 1:2], scalar2=None, op0=ALU.mult)
            cqo = small.tile([128, 1], f32, tag="cqo")
            nc.vector.reciprocal(out=cqo, in_=amo)
            nc.vector.tensor_scalar_mul(out=cqo, in0=cqo, scalar1=127.0)
            qsc = scratch.tile([128, D], f32, tag="qsc")
            nc.vector.tensor_scalar(out=qsc, in0=o_all[:, tt, :], scalar1=cqo,
                                    scalar2=MAGIC, op0=ALU.mult, op1=ALU.add)
            oq = xqpool.tile([128, D], bf16, tag="oq")
            nc.vector.tensor_scalar_add(out=oq, in0=qsc, scalar1=-MAGIC)
            pe_transpose(oq, oqT, tt * 128, ident_bf, bf16)

        for tt in range(2):
            pOut = psA.tile([128, 512], f32, tag="pk")
            for j in range(4):
                nc.tensor.matmul(pOut, lhsT=oqT[:, j, tt * 128 : (tt + 1) * 128],
                                 rhs=wo4[:, j, :], start=(j == 0), stop=(j == 3))
            ysb = ypool.tile([128, D], f32, tag="ysb")
            nc.scalar.activation(out=ysb, in_=pOut, func=ACTF.Copy,
                                 scale=osc_all[:, tt : tt + 1])
            xsk = xpool.tile([128, D], f16, tag="xt")
            nc.sync.dma_start(out=xsk, in_=d_xo[tt * 128 : (tt + 1) * 128, :])
            nc.vector.tensor_add(ysb, ysb, xsk)
            nc.sync.dma_start(out=d_y[tt * 128 : (tt + 1) * 128, :], in_=ysb)

    nc.compile()
    return nc


def _host_prep(x, pos, cond, ada_w, qkv_w, scale, out_w):
    x = np.asarray(x, np.float32).reshape(NTOK, D)
    pos = np.asarray(pos, np.float32).reshape(NTOK, 3)
    cond = np.asarray(cond, np.float32).reshape(D)
    ada_w = np.asarray(ada_w, np.float32)
    qkv_w = np.asarray(qkv_w, np.float32)
    scale = np.asarray(scale, np.float32).reshape(NH)
    out_w = np.asarray(out_w, np.float32)

    sw1 = 1.0 / max(np.mean(np.abs(qkv_w)), 1e-5)
    wt1 = np.clip(np.round(qkv_w * sw1), -1, 1).astype(np.float32)  # [1536, 512]
    sw2 = 1.0 / max(np.mean(np.abs(out_w)), 1e-5)
    wt2 = np.clip(np.round(out_w * sw2), -1, 1).astype(np.float32)  # [512, 512]

    cs2, sn2 = _rope_tables(pos)
    masks = _make_masks()

    prep = {
        "x16": x.astype(F16),
        "cs2": cs2, "sn2": sn2, "masks": masks,
        "wkv": np.ascontiguousarray(wt1[512:, :].T).astype(BF16),  # [512, 1024]
        "wq": np.ascontiguousarray(wt1[:512, :].T).astype(BF16),   # [512, 512]
        "wo": np.ascontiguousarray(wt2.T).astype(BF16),            # [512, 512]
        "adwt": np.ascontiguousarray(ada_w.T).astype(F16),         # [512, 512]
        "cnd": np.ascontiguousarray(cond.reshape(4, 128).T).astype(F16),  # [128, 4]
        "scl": scale.reshape(1, NH).astype(np.float32),
        "kon": np.array([[1.0 / (127.0 * sw1), 1.0 / (127.0 * sw2)]], np.float32),
    }
    return prep


def _in_maps(prep):
    maps = []
    for c in range(8):
        tlo = min(max(c - 1, 0), T - KT)
        halo = slice(tlo * PLANE, (tlo + 3) * PLANE)
        own = slice(c * PLANE, (c + 1) * PLANE)
        maps.append({
            "xh": np.ascontiguousarray(prep["x16"][halo]),
            "xo": np.ascontiguousarray(prep["x16"][own]),
            "csh": np.ascontiguousarray(prep["cs2"][halo]),
            "snh": np.ascontiguousarray(prep["sn2"][halo]),
            "cso": np.ascontiguousarray(prep["cs2"][own]),
            "sno": np.ascontiguousarray(prep["sn2"][own]),
            "msk": prep["masks"],
            "wkv": prep["wkv"], "wq": prep["wq"], "wo": prep["wo"],
            "adwt": prep["adwt"], "cnd": prep["cnd"],
            "scl": prep["scl"], "kon": prep["kon"],
        })
    return maps


def _get_program():
    if "nc" not in _CACHE:
        _CACHE["nc"] = _build_program()
    return _CACHE["nc"]


def kernel(x, pos, cond, ada_w, qkv_w, scale, out_w):
    from concourse.bass_utils import run_bass_kernel_spmd

    nc = _get_program()
    prep = _host_prep(x, pos, cond, ada_w, qkv_w, scale, out_w)
    maps = _in_maps(prep)
    trace = bool(int(os.environ.get("KERNEL_TRACE", "0")))
    kwargs = {}
    if trace:
        kwargs["trace"] = True
        td = os.environ.get("KERNEL_TRACE_DIR")
        if td:
            import tempfile

            kwargs["tmpdir"] = tempfile.mkdtemp(dir=td)
    res = run_bass_kernel_spmd(nc, maps, core_ids=list(range(8)), **kwargs)
    _CACHE["last_exec_time_ns"] = res.exec_time_ns
    out = np.concatenate([res.results[c]["y"] for c in range(8)], axis=0)
    return out.reshape(1, T, H, W, D).astype(np.float32)



# revision 10
# speedup vs baseline: 1.0648x; 1.0648x over previous
"""Trainium2 Bass kernel for nn_NeighborhoodSelfAttentionBlock.

Strategy (8 NeuronCores, single launch, SPMD):
  - Shard the T axis: core c computes the output for T-plane c (256 tokens).
  - Each core redundantly preprocesses + projects qkv for its 3-plane halo
    (clamped NATTEN window), so no cross-core communication is needed.
  - BitLinear is computed exactly: int8-grid activations and ternary weights
    are exact in bf16; the matmul accumulates exact integers in f32 PSUM.
    Rounding uses the f32 magic-number trick (round-half-even == jnp.round).
  - Cosine-sim attention is scale invariant, so q/k stay in integer scale
    until normalization; softmax needs no max-subtraction (|logits| <= 10).
  - 3D neighborhood attention: 4-row query strips x (3 t-planes) key blocks,
    block-dense logits in L^T layout (keys on partitions) with host-built
    masks applied multiplicatively after exp; denominator via a ones column
    appended to v.
  - rsqrt is computed as exp(-0.5*ln(x)) so the single ACT table set
    natural_log_exp_and_others covers every activation in the kernel.
"""

import math
import os
import sys

import numpy as np

sys.path.insert(0, "/opt/trn_rl_repo")

import ml_dtypes

BF16 = ml_dtypes.bfloat16
F16 = np.float16

D = 512
NH = 8
DH = 64
KT, KH, KW = 3, 5, 5
T, H, W = 8, 16, 16
NTOK = T * H * W
PLANE = H * W  # 256
MAGIC = float(np.float32(1.5 * 2 ** 23))
EPS = 1e-6

# attention strip geometry: 4 query h-rows per strip; key blocks are the
# half-planes (128 tokens) overlapping the strip's h-window, per t-plane.
HALVES = [[0], [0, 1], [0, 1], [1]]

_CACHE = {}


def _win_starts(n, k):
    return np.clip(np.arange(n) - k // 2, 0, n - k)


def _make_masks():
    hs = _win_starts(H, KH)
    ws = _win_starts(W, KW)
    big = np.zeros((2, 128, 192), np.float16)
    for eta in range(2):
        strips = [0, 1, 2] if eta == 0 else [1, 2, 3]
        for si, s in enumerate(strips):
            for i, h in enumerate(range(4 * s, 4 * s + 4)):
                for w in range(W):
                    for hk in range(hs[h], hs[h] + KH):
                        if not (8 * eta <= hk < 8 * eta + 8):
                            continue
                        for wk in range(ws[w], ws[w] + KW):
                            big[eta, (hk - 8 * eta) * W + wk,
                                si * 64 + i * W + w] = 1.0
    return big


def _rope_tables(pos):
    dim = DH // 4
    npgh = dim // 4
    freqs = np.exp(
        np.linspace(math.log(math.pi), math.log(10 * math.pi), NH * npgh + 1)[:-1]
    )
    freqs = freqs.reshape(npgh, NH).T  # (8, 4)
    theta = np.concatenate(
        [pos[:, None, a : a + 1] * freqs[None, :, :] for a in range(3)], axis=-1
    ).astype(np.float32)  # (tok, 8, 12)
    cos, sin = np.cos(theta), np.sin(theta)
    cs2 = np.concatenate([cos, cos], axis=-1).astype(F16)  # (tok, 8, 24)
    sn2 = np.concatenate([-sin, sin], axis=-1).astype(F16)
    return cs2.reshape(NTOK, NH * 24), sn2.reshape(NTOK, NH * 24)


def _make_bacc_class():
    import bass_rust as _bass_rust
    import concourse.bacc as bacc
    from concourse import mybir
    from concourse.hw_specs import get_activation_tables

    class _Bacc(bacc.Bacc):
        """Bacc that pins every activation to natural_log_exp_and_others
        (covers exp/ln/square/copy/identity) so only one ACT table load is
        emitted instead of thrashing between per-function default sets."""

        _KEEP = "natural_log_exp_and_others"

        def insert_act_table_loads(self):
            has_activation = any(
                isinstance(i, mybir.InstActivation)
                for b in self.main_func.blocks
                for i in b.instructions
            )
            if not has_activation:
                return
            used = {
                i.func
                for b in self.main_func.blocks
                for i in b.instructions
                if isinstance(i, mybir.InstActivation)
            }
            all_tables = get_activation_tables(self.m.arch)
            keep_fns = all_tables.get(self._KEEP, set())
            subtract = used & keep_fns
            tables = []
            for name, fns in all_tables.items():
                if name != self._KEEP:
                    fns = fns - subtract
                tables.append((name, fns))
            _bass_rust.insert_act_table_loads(self, tables)

    return _Bacc


def _build_program():
    import concourse.bacc as bacc
    import concourse.bass as bass
    import concourse.tile as tile
    from concourse import mybir

    f32, f16, bf16 = mybir.dt.float32, mybir.dt.float16, mybir.dt.bfloat16
    AX = mybir.AxisListType
    ALU = mybir.AluOpType
    ACTF = mybir.ActivationFunctionType

    nc = _make_bacc_class()("TRN2", target_bir_lowering=False, debug=False, num_devices=8)

    # ---- DRAM I/O ----
    d_xh = nc.dram_tensor("xh", [3 * PLANE, D], f16, kind="ExternalInput")
    d_xo = nc.dram_tensor("xo", [PLANE, D], f16, kind="ExternalInput")
    d_csh = nc.dram_tensor("csh", [3 * PLANE, NH * 24], f16, kind="ExternalInput")
    d_snh = nc.dram_tensor("snh", [3 * PLANE, NH * 24], f16, kind="ExternalInput")
    d_cso = nc.dram_tensor("cso", [PLANE, NH * 24], f16, kind="ExternalInput")
    d_sno = nc.dram_tensor("sno", [PLANE, NH * 24], f16, kind="ExternalInput")
    d_msk = nc.dram_tensor("msk", [2, 128, 192], f16, kind="ExternalInput")
    d_wkv = nc.dram_tensor("wkv", [D, 1024], bf16, kind="ExternalInput")
    d_wq = nc.dram_tensor("wq", [D, 512], bf16, kind="ExternalInput")
    d_wo = nc.dram_tensor("wo", [D, 512], bf16, kind="ExternalInput")
    d_adwt = nc.dram_tensor("adwt", [D, D], f16, kind="ExternalInput")
    d_cnd = nc.dram_tensor("cnd", [128, 4], f16, kind="ExternalInput")
    d_scl = nc.dram_tensor("scl", [1, NH], f32, kind="ExternalInput")
    d_kon = nc.dram_tensor("kon", [1, 2], f32, kind="ExternalInput")
    d_y = nc.dram_tensor("y", [PLANE, D], f32, kind="ExternalOutput")

    from contextlib import ExitStack
    with tile.TileContext(nc) as tc, ExitStack() as ctx:
        consts = ctx.enter_context(tc.tile_pool(name="consts", bufs=1))
        wpool = ctx.enter_context(tc.tile_pool(name="wpool", bufs=1))
        xpool = ctx.enter_context(tc.tile_pool(name="xpool", bufs=6))
        xapool = ctx.enter_context(tc.tile_pool(name="xapool", bufs=8))
        scratch = ctx.enter_context(tc.tile_pool(name="scratch", bufs=3))
        stats = ctx.enter_context(tc.tile_pool(name="stats", bufs=1))
        xqpool = ctx.enter_context(tc.tile_pool(name="xqpool", bufs=3))
        persist = ctx.enter_context(tc.tile_pool(name="persist", bufs=1))
        kqpool = ctx.enter_context(tc.tile_pool(name="kqpool", bufs=3))
        attp = ctx.enter_context(tc.tile_pool(name="attp", bufs=24))
        small = ctx.enter_context(tc.tile_pool(name="small", bufs=4))
        ypool = ctx.enter_context(tc.tile_pool(name="ypool", bufs=2))
        psA = ctx.enter_context(tc.tile_pool(name="psA", bufs=2, space="PSUM"))
        psL = ctx.enter_context(tc.tile_pool(name="psL", bufs=2, space="PSUM"))
        psO = ctx.enter_context(tc.tile_pool(name="psO", bufs=2, space="PSUM"))

        # ---- constants / weights ----
        # DMA issue order is critical-path order: adas deps + x tiles first,
        # then per-phase weights/tables just ahead of their consumers.
        adas_bc = consts.tile([128, D], f32)
        scale_bc = consts.tile([128, NH], f32)
        kon_bc = consts.tile([128, 2], f32)
        masks_t = consts.tile([128, 2, 192], f16)
        eps_ap = consts.tile([128, 1], f32)
        cnd_t = consts.tile([128, 4], f16)
        ones_t = consts.tile([1, 128], f16)
        nc.vector.memset(eps_ap, EPS)
        nc.vector.memset(ones_t, 1.0)

        adwt4 = wpool.tile([128, 4, 512], f16)
        nc.sync.dma_start(out=cnd_t, in_=d_cnd[:, :])
        nc.sync.dma_start(out=adwt4, in_=d_adwt.ap().rearrange("(j p) n -> p j n", p=128))
        nc.sync.dma_start(out=scale_bc, in_=d_scl.ap().broadcast_to([128, NH]))
        nc.sync.dma_start(out=kon_bc, in_=d_kon.ap().broadcast_to([128, 2]))

        xt_tiles = []
        for i in range(8):
            own = i >= 6
            src = d_xo if own else d_xh
            row = (i - 6) * 128 if own else i * 128
            if own:
                xt = persist.tile([128, D], f16, name=f"xown{i - 6}")
            else:
                xt = xpool.tile([128, D], f16, tag="xt")
            nc.sync.dma_start(out=xt, in_=src[row : row + 128, :])
            xt_tiles.append(xt)

        wkv4 = wpool.tile([128, 4, 1024], bf16)
        wq4 = wpool.tile([128, 4, 512], bf16)
        wo4 = wpool.tile([128, 4, 512], bf16)
        nc.sync.dma_start(out=wkv4, in_=d_wkv.ap().rearrange("(j p) n -> p j n", p=128))

        csh_t = persist.tile([128, 6, NH * 24], f16)
        snh_t = persist.tile([128, 6, NH * 24], f16)
        cso_t = persist.tile([128, 2, NH * 24], f16)
        sno_t = persist.tile([128, 2, NH * 24], f16)
        nc.sync.dma_start(out=csh_t, in_=d_csh.ap().rearrange("(i p) n -> p i n", p=128))
        nc.sync.dma_start(out=snh_t, in_=d_snh.ap().rearrange("(i p) n -> p i n", p=128))
        nc.sync.dma_start(out=wq4, in_=d_wq.ap().rearrange("(j p) n -> p j n", p=128))
        nc.sync.dma_start(out=cso_t, in_=d_cso.ap().rearrange("(i p) n -> p i n", p=128))
        nc.sync.dma_start(out=sno_t, in_=d_sno.ap().rearrange("(i p) n -> p i n", p=128))
        nc.sync.dma_start(out=masks_t, in_=d_msk.ap().rearrange("s p q -> p s q"))
        nc.sync.dma_start(out=wo4, in_=d_wo.ap().rearrange("(j p) n -> p j n", p=128))

        def xbar_transpose(src, dstT, col):
            # dstT[:, j, col:col+128] <- src[:, j*128:(j+1)*128].T via DMA xbar
            nc.sync.dma_start(out=dstT[:, :, col : col + 128], in_=src,
                              transpose=True)

        # ---- adas = cond @ ada_w.T + 1, broadcast to 128 partitions ----
        pad = psO.tile([1, D], f32, tag="pO")
        for j in range(4):
            nc.tensor.matmul(pad, lhsT=cnd_t[:, j : j + 1], rhs=adwt4[:, j, :],
                             start=(j == 0), stop=(j == 3))
        ad1 = small.tile([1, D], f16)
        nc.scalar.activation(out=ad1, in_=pad, func=ACTF.Identity, bias=1.0, scale=1.0)
        pad2 = psO.tile([128, D], f32, tag="pO")
        nc.tensor.matmul(pad2, lhsT=ones_t, rhs=ad1, start=True, stop=True)
        nc.vector.tensor_copy(out=adas_bc, in_=pad2)

        # ---- preprocess + quantize x (6 halo tiles, then 2 own tiles) ----
        ss_all = stats.tile([128, 8], f32)
        am_all = stats.tile([128, 8], f32)
        xa_tiles = []
        for i in range(8):
            xt = xt_tiles[i]
            sq = scratch.tile([128, D], f16, tag="sq")
            nc.scalar.activation(out=sq, in_=xt, func=ACTF.Square,
                                 accum_out=ss_all[:, i : i + 1])
            xa = xapool.tile([128, D], f16, tag="xa")
            nc.gpsimd.tensor_mul(xa, xt, adas_bc)
            nc.vector.reduce_max(out=am_all[:, i : i + 1], in_=xa, axis=AX.X,
                                 apply_absolute_value=True)
            xa_tiles.append(xa)

        lnv = stats.tile([128, 8], f32)
        nc.scalar.activation(out=lnv, in_=ss_all, func=ACTF.Ln, bias=eps_ap,
                             scale=1.0 / D)
        rstd = stats.tile([128, 8], f32)
        nc.scalar.activation(out=rstd, in_=lnv, func=ACTF.Exp, bias=0.0, scale=-0.5)
        hm = stats.tile([128, 8], f32)
        nc.vector.tensor_mul(hm, rstd, am_all)
        nc.vector.tensor_scalar_max(out=hm, in0=hm, scalar1=1e-5)
        vs_all = stats.tile([128, 8], f32)
        nc.vector.tensor_scalar(out=vs_all, in0=hm, scalar1=kon_bc[:, 0:1],
                                scalar2=None, op0=ALU.mult)
        hm127 = stats.tile([128, 8], f32)
        nc.vector.tensor_scalar_mul(out=hm127, in0=hm, scalar1=1.0 / 127.0)
        rec = stats.tile([128, 8], f32)
        nc.vector.reciprocal(out=rec, in_=hm127)
        cq_all = stats.tile([128, 8], f32)
        nc.vector.tensor_mul(cq_all, rec, rstd)

        xqT = persist.tile([128, 4, 6 * 128], bf16)   # halo x_q^T
        xqoT = persist.tile([128, 4, 2 * 128], bf16)  # own  x_q^T
        for i in range(8):
            own = i >= 6
            qsc = scratch.tile([128, D], f32, tag="qsc")
            nc.vector.tensor_scalar(out=qsc, in0=xa_tiles[i],
                                    scalar1=cq_all[:, i : i + 1], scalar2=MAGIC,
                                    op0=ALU.mult, op1=ALU.add)
            xq = xqpool.tile([128, D], bf16, tag="xq")
            nc.vector.tensor_scalar_add(out=xq, in0=qsc, scalar1=-MAGIC)
            dstT = xqoT if own else xqT
            col = (i - 6) * 128 if own else i * 128
            xbar_transpose(xq, dstT, col)

        # ---- kv projection + k/v postprocessing (6 halo chunks) ----
        kT = persist.tile([128, 4, 6 * 128], f16)
        v_sb = persist.tile([128, 6, NH * 65], f16)
        # ones columns for the denominator
        nc.vector.memset(v_sb, 1.0)

        def rope_norm(psum, i, cs_t, sn_t, dstT, dst_col, is_q):
            """psum [128,512] int-valued q/k; rope + normalize -> dstT via DMA-T."""
            z = kqpool.tile([128, NH, DH], f16, tag="z")
            # pass-through dims 24:64
            nc.scalar.activation(out=z[:, :, 24:DH],
                                 in_=psum.rearrange("p (h d) -> p h d", h=NH)[:, :, 24:DH],
                                 func=ACTF.Copy)
            rot = psum.rearrange("p (h d) -> p h d", h=NH)[:, :, 0:24]
            m1 = kqpool.tile([128, NH, 24], f16, tag="m1")
            nc.vector.tensor_mul(m1, rot, cs_t[:, i, :].rearrange("p (h d) -> p h d", h=NH))
            swap = bass.AP(tensor=rot.tensor, offset=rot.offset + 12,
                           ap=[list(rot.ap[0]), list(rot.ap[1]), [-12, 2], [1, 12]])
            m2 = kqpool.tile([128, NH, 2, 12], f16, tag="m2")
            nc.vector.tensor_mul(
                m2, swap,
                sn_t[:, i, :].rearrange("p (h two tw) -> p h two tw", h=NH, two=2))
            m2 = m2[:, :, :, :].rearrange("p h two tw -> p h (two tw)")
            nc.vector.tensor_add(z[:, :, 0:24], m1, m2)
            # norms per (token, head)
            zsq = scratch.tile([128, NH, DH], f32, tag="zsq")
            nc.vector.tensor_mul(zsq, z, z)
            ssz = small.tile([128, NH], f32, tag="ssz")
            nc.vector.reduce_sum(out=ssz, in_=zsq, axis=AX.X)
            lnz = small.tile([128, NH], f32, tag="lnz")
            nc.scalar.activation(out=lnz, in_=ssz, func=ACTF.Ln, bias=eps_ap, scale=1.0)
            rs0 = small.tile([128, NH], f32, tag="rs0")
            nc.scalar.activation(out=rs0, in_=lnz, func=ACTF.Exp, bias=0.0, scale=-0.5)
            if is_q:
                nc.vector.tensor_mul(rs0, rs0, scale_bc)
            rs16 = small.tile([128, NH], f16, tag="rs16")
            nc.vector.tensor_copy(out=rs16, in_=rs0)
            zn = kqpool.tile([128, NH, DH], f16, tag="zn")
            nc.vector.tensor_mul(zn, z, rs16[:, :, None].broadcast_to([128, NH, DH]))
            znf = zn.rearrange("p h d -> p (h d)")
            xbar_transpose(znf, dstT, dst_col)

        for i in range(6):
            pk = psA.tile([128, 512], f32, tag="pk")
            for j in range(4):
                nc.tensor.matmul(pk, lhsT=xqT[:, j, i * 128 : (i + 1) * 128],
                                 rhs=wkv4[:, j, 0:512], start=(j == 0), stop=(j == 3))
            pv = psA.tile([128, 512], f32, tag="pv")
            for j in range(4):
                nc.tensor.matmul(pv, lhsT=xqT[:, j, i * 128 : (i + 1) * 128],
                                 rhs=wkv4[:, j, 512:1024], start=(j == 0), stop=(j == 3))
            rope_norm(pk, i, csh_t, snh_t, kT, i * 128, is_q=False)
            nc.scalar.activation(
                out=v_sb[:, i, :].rearrange("p (h d) -> p h d", h=NH)[:, :, 0:DH],
                in_=pv.rearrange("p (h d) -> p h d", h=NH),
                func=ACTF.Copy, scale=vs_all[:, i : i + 1])

        # ---- q projection + postprocessing (2 own chunks) ----
        qnT = persist.tile([128, 4, 2 * 128], f16)
        for i in range(2):
            pq = psA.tile([128, 512], f32, tag="pk")
            for j in range(4):
                nc.tensor.matmul(pq, lhsT=xqoT[:, j, i * 128 : (i + 1) * 128],
                                 rhs=wq4[:, j, :], start=(j == 0), stop=(j == 3))
            rope_norm(pq, i, cso_t, sno_t, qnT, i * 128, is_q=True)

        # ---- neighborhood attention ----
        # Batched QK: one matmul per (head, t-plane, half-plane) covering the
        # 3 query strips that use that key block (contiguous query columns).
        o_all = persist.tile([128, 2, D], f16)
        for half in range(2):
            PTs = {}
            for hh in range(4):
                h = half * 4 + hh
                hp, hc = 64 * (h % 2), h // 2
                for ti in range(3):
                    for eta in range(2):
                        pLt = psL.tile([128, 192], f32, tag="pL")
                        nc.tensor.matmul(
                            pLt,
                            lhsT=kT[hp : hp + 64, hc,
                                    ti * 256 + eta * 128 : ti * 256 + eta * 128 + 128],
                            rhs=qnT[hp : hp + 64, hc, eta * 64 : eta * 64 + 192],
                            start=True, stop=True)
                        PT = attp.tile([128, 192], f16, tag="PT")
                        nc.scalar.activation(out=PT, in_=pLt, func=ACTF.Exp)
                        nc.gpsimd.tensor_mul(PT, PT, masks_t[:, eta, :])
                        PTs[(hh, ti, eta)] = PT
            for s in range(4):
                pO = psO.tile([64, 4 * 65], f32, tag="pO")
                for hh in range(4):
                    h = half * 4 + hh
                    blocks = [(ti, eta) for ti in range(3) for eta in HALVES[s]]
                    for bi, (ti, eta) in enumerate(blocks):
                        qoff = (s - eta) * 64
                        nc.tensor.matmul(
                            pO[:, hh * 65 : (hh + 1) * 65],
                            lhsT=PTs[(hh, ti, eta)][:, qoff : qoff + 64],
                            rhs=v_sb[:, 2 * ti + eta, h * 65 : (h + 1) * 65],
                            start=(bi == 0), stop=(bi == len(blocks) - 1))
                recd = small.tile([64, 4], f32, tag="recd")
                den = bass.AP(tensor=pO.tensor, offset=pO.offset + 64,
                              ap=[list(pO.ap[0]), [65, 4]])
                nc.vector.reciprocal(out=recd, in_=den)
                num = bass.AP(tensor=pO.tensor, offset=pO.offset,
                              ap=[list(pO.ap[0]), [65, 4], [1, 64]])
                nc.vector.tensor_mul(
                    o_all[(s % 2) * 64 : (s % 2) * 64 + 64, s // 2,
                          half * 256 : half * 256 + 256].rearrange(
                              "p (a b) -> p a b", a=4),
                    num, recd[:, :, None].broadcast_to([64, 4, 64]))

        # ---- out projection (BitLinear) + residual ----
        oqT = persist.tile([128, 4, 2 * 128], bf16)
        osc_all = stats.tile([128, 2], f32)
        for tt in range(2):
            amo = small.tile([128, 1], f32, tag="amo")
            nc.vector.reduce_max(out=amo, in_=o_all[:, tt, :], axis=AX.X,
                                 apply_absolute_value=True)
            nc.vector.tensor_scalar_max(out=amo, in0=amo, scalar1=1e-5)
            nc.vector.tensor_scalar(out=osc_all[:, tt : tt + 1], in0=amo,
                                    scalar1=kon_bc[:, 1:2], scalar2=None, op0=ALU.mult)
            cqo = small.tile([128, 1], f32, tag="cqo")
            nc.vector.reciprocal(out=cqo, in_=amo)
            nc.vector.tensor_scalar_mul(out=cqo, in0=cqo, scalar1=127.0)
            qsc = scratch.tile([128, D], f32, tag="qsc")
            nc.vector.tensor_scalar(out=qsc, in0=o_all[:, tt, :], scalar1=cqo,
                                    scalar2=MAGIC, op0=ALU.mult, op1=ALU.add)
            oq = xqpool.tile([128, D], bf16, tag="oq")
            nc.vector.tensor_scalar_add(out=oq, in0=qsc, scalar1=-MAGIC)
            xbar_transpose(oq, oqT, tt * 128)

        for tt in range(2):
            pOut = psA.tile([128, 512], f32, tag="pk")
            for j in range(4):
                nc.tensor.matmul(pOut, lhsT=oqT[:, j, tt * 128 : (tt + 1) * 128],
                                 rhs=wo4[:, j, :], start=(j == 0), stop=(j == 3))
            ysb = ypool.tile([128, D], f32, tag="ysb")
            nc.scalar.activation(out=ysb, in_=pOut, func=ACTF.Copy,
                                 scale=osc_all[:, tt : tt + 1])
            nc.vector.tensor_add(ysb, ysb, xt_tiles[6 + tt])
            nc.sync.dma_start(out=d_y[tt * 128 : (tt + 1) * 128, :], in_=ysb)

    nc.compile()
    return nc


def _host_prep(x, pos, cond, ada_w, qkv_w, scale, out_w):
    x = np.asarray(x, np.float32).reshape(NTOK, D)
    pos = np.asarray(pos, np.float32).reshape(NTOK, 3)
    cond = np.asarray(cond, np.float32).reshape(D)
    ada_w = np.asarray(ada_w, np.float32)
    qkv_w = np.asarray(qkv_w, np.float32)
    scale = np.asarray(scale, np.float32).reshape(NH)
    out_w = np.asarray(out_w, np.float32)

    sw1 = 1.0 / max(np.mean(np.abs(qkv_w)), 1e-5)
    wt1 = np.clip(np.round(qkv_w * sw1), -1, 1).astype(np.float32)  # [1536, 512]
    sw2 = 1.0 / max(np.mean(np.abs(out_w)), 1e-5)
    wt2 = np.clip(np.round(out_w * sw2), -1, 1).astype(np.float32)  # [512, 512]

    cs2, sn2 = _rope_tables(pos)
    masks = _make_masks()

    prep = {
        "x16": x.astype(F16),
        "cs2": cs2, "sn2": sn2, "masks": masks,
        "wkv": np.ascontiguousarray(wt1[512:, :].T).astype(BF16),  # [512, 1024]
        "wq": np.ascontiguousarray(wt1[:512, :].T).astype(BF16),   # [512, 512]
        "wo": np.ascontiguousarray(wt2.T).astype(BF16),            # [512, 512]
        "adwt": np.ascontiguousarray(ada_w.T).astype(F16),         # [512, 512]
        "cnd": np.ascontiguousarray(cond.reshape(4, 128).T).astype(F16),  # [128, 4]
        "scl": scale.reshape(1, NH).astype(np.float32),
        "kon": np.array([[1.0 / (127.0 * sw1), 1.0 / (127.0 * sw2)]], np.float32),
    }
    return prep


def _in_maps(prep):
    maps = []
    for c in range(8):
        tlo = min(max(c - 1, 0), T - KT)
        halo = slice(tlo * PLANE, (tlo + 3) * PLANE)
        own = slice(c * PLANE, (c + 1) * PLANE)
        maps.append({
            "xh": np.ascontiguousarray(prep["x16"][halo]),
            "xo": np.ascontiguousarray(prep["x16"][own]),
            "csh": np.ascontiguousarray(prep["cs2"][halo]),
            "snh": np.ascontiguousarray(prep["sn2"][halo]),
            "cso": np.ascontiguousarray(prep["cs2"][own]),
            "sno": np.ascontiguousarray(prep["sn2"][own]),
            "msk": prep["masks"],
            "wkv": prep["wkv"], "wq": prep["wq"], "wo": prep["wo"],
            "adwt": prep["adwt"], "cnd": prep["cnd"],
            "scl": prep["scl"], "kon": prep["kon"],
        })
    return maps


def _get_program():
    if "nc" not in _CACHE:
        _CACHE["nc"] = _build_program()
    return _CACHE["nc"]


def kernel(x, pos, cond, ada_w, qkv_w, scale, out_w):
    from concourse.bass_utils import run_bass_kernel_spmd

    nc = _get_program()
    prep = _host_prep(x, pos, cond, ada_w, qkv_w, scale, out_w)
    maps = _in_maps(prep)
    trace = bool(int(os.environ.get("KERNEL_TRACE", "0")))
    kwargs = {}
    if trace:
        kwargs["trace"] = True
        td = os.environ.get("KERNEL_TRACE_DIR")
        if td:
            import tempfile

            kwargs["tmpdir"] = tempfile.mkdtemp(dir=td)
    res = run_bass_kernel_spmd(nc, maps, core_ids=list(range(8)), **kwargs)
    _CACHE["last_exec_time_ns"] = res.exec_time_ns
    out = np.concatenate([res.results[c]["y"] for c in range(8)], axis=0)
    return out.reshape(1, T, H, W, D).astype(np.float32)

